# revision 20
# baseline (speedup 1.0000x reference)
"""Trainium2 Bass kernel for nn_Attr_Tokenizer (retrieval_knn).

For N=131072 samples: center (x-y), rotate into the anchor frame, find the
nearest of the 2821 fixed disk-grid points, return (index int32 [N],
offset [N,2]).

Exact closed form (no 2821-wide scan). Per sample, 12 candidates provably
cover the argmin (validated on 10M-sample sweeps + the real data):
  A     = (round(rx), clamp(round(ry), +-xmax(round(rx))))   [interior opt]
  rows  = 5 rows around the disk-projection row; per-row optimum is
          (sign(rx)*min(|round(rx)|, xmax(r)), r)
  hull  = 6 convex-hull vertices of the lattice-disk boundary (mirrored)
xmax(r) = floor(sqrt(900-r^2)) computed exactly: t = round(ACT_sqrt(s));
xm = t - (t*t > s)  -- tolerant of the loose ACT sqrt LUT.
index = base[i] + xmax + gx with i = 30-gy; base[] evaluated by an
exact-rounding degree-14 polynomial on the half domain + reflection.

fp32 simulation vs the jax reference: 1/131072 index mismatch (an fp32
sqrt-collapse tie), offset rel err ~1e-3.

Implementation: raw Bass (no Tile) - serial engine ping-pong with
single-semaphore waits. Data-parallel over 8 NeuronCores (shard N,
no cross-core communication).
"""
import numpy as np

N = 131072
NCORES = 8
NS = N // NCORES       # 16384 per core
P = 128
F = NS // P            # 128

MAGIC = 12582912.0     # 1.5*2^23: (x+M)-M == round-to-nearest for |x| < 2^22

# deg-14 power-basis coefficients in v=(u-15)/15 for base[u], u=0..30
# (discrete minimax fit, max residual 0.441; fp32 Horner + round is exact)
POLY = [526.652099609375, 767.9715576171875, 131.24649047851562,
        27.565488815307617, 32.190269470214844, -676.077880859375,
        -417.0867004394531, 2375.08642578125, 2140.177734375,
        -4234.26904296875, -4233.40185546875, 3640.740234375,
        3674.610107421875, -1211.4583740234375, -1164.38818359375]

HULL = [(29.0, 7.0), (27.0, 13.0), (24.0, 18.0),
        (18.0, 24.0), (13.0, 27.0), (7.0, 29.0)]

_CACHE = {}


def _build():
    import concourse.bass as bass
    from concourse import mybir

    f32 = mybir.dt.float32
    u8 = mybir.dt.uint8
    A = mybir.AluOpType
    ACT = mybir.ActivationFunctionType
    HALFPI = float(np.pi / 2)

    nc = bass.Bass()
    inpd = nc.declare_dram_parameter("inp", [P, 5 * F], f32, isOutput=False)
    outd = nc.declare_dram_parameter("outp", [P, 3 * F], f32, isOutput=True)

    tiles = {}

    def T(name, w=F, dt=None):
        if name not in tiles:
            tiles[name] = nc.alloc_sbuf_tensor(name, [P, w], dt or f32)
        return tiles[name][:]

    inp = T("inp_t", 5 * F)
    outt = T("outt", 3 * F)
    x0 = tiles["inp_t"][:, 0:F]
    x1 = tiles["inp_t"][:, F:2 * F]
    y0 = tiles["inp_t"][:, 2 * F:3 * F]
    y1 = tiles["inp_t"][:, 3 * F:4 * F]
    th = tiles["inp_t"][:, 4 * F:5 * F]

    # linear program: list of (engine, fn) steps; engines alternate with
    # a semaphore ping-pong so every instruction carries <= 1 wait.
    prog = []  # ("V"|"S", callable(engine))

    def V(fn):
        prog.append(("V", fn))

    def S(fn):
        prog.append(("S", fn))

    # ---------- constant tiles
    V(lambda e: e.memset(T("ZERO"), 0.0))
    V(lambda e: e.memset(T("C30"), 30.0))
    V(lambda e: e.memset(T("CN30"), -30.0))
    V(lambda e: e.memset(T("C28"), 28.0))
    V(lambda e: e.memset(T("CN28"), -28.0))
    V(lambda e: e.memset(T("EPS"), 1e-6))
    V(lambda e: e.memset(T("CPI"), float(np.pi)))

    # ---------- phase 1: center, theta prep
    V(lambda e: e.tensor_sub(T("cx0"), x0, y0))
    V(lambda e: e.tensor_sub(T("cy0"), x1, y1))
    V(lambda e: e.tensor_scalar(T("th2"), th, HALFPI, None, A.add))
    V(lambda e: e.tensor_tensor(T("thm", dt=u8), T("th2"), T("CPI"), A.is_gt))
    V(lambda e: e.tensor_copy(T("thmf"), T("thm", dt=u8)))
    V(lambda e: e.scalar_tensor_tensor(T("th2w"), T("thmf"), float(-2.0 * np.pi), T("th2"), A.mult, A.add))
    # ---------- sin/cos
    S(lambda e: e.activation(T("c"), th, ACT.Sin))
    S(lambda e: e.activation(T("s"), T("th2w"), ACT.Sin))
    # ---------- rotate (same fp32 op order as the reference)
    V(lambda e: e.tensor_mul(T("m1"), T("cx0"), T("c")))
    V(lambda e: e.tensor_mul(T("m2"), T("cy0"), T("s")))
    V(lambda e: e.tensor_mul(T("m3"), T("cx0"), T("s")))
    V(lambda e: e.tensor_mul(T("m4"), T("cy0"), T("c")))
    V(lambda e: e.tensor_sub(T("rx"), T("m1"), T("m2")))
    V(lambda e: e.tensor_add(T("ry"), T("m3"), T("m4")))
    # rounds / abs / signs
    V(lambda e: e.tensor_scalar(T("rnd0"), T("rx"), MAGIC, MAGIC, A.add, A.subtract))
    V(lambda e: e.tensor_tensor(T("rnd1"), T("rnd0"), T("CN30"), A.max))
    V(lambda e: e.tensor_tensor(T("rndx"), T("rnd1"), T("C30"), A.min))
    V(lambda e: e.tensor_scalar(T("rndy"), T("ry"), MAGIC, MAGIC, A.add, A.subtract))
    V(lambda e: e.tensor_scalar(T("nrndx"), T("rndx"), -1.0, None, A.mult))
    V(lambda e: e.tensor_tensor(T("arnd"), T("rndx"), T("nrndx"), A.max))
    V(lambda e: e.tensor_tensor(T("sx0"), T("rx"), T("ZERO"), A.is_ge))
    V(lambda e: e.tensor_scalar(T("sxp"), T("sx0"), 2.0, -1.0, A.mult, A.add))
    V(lambda e: e.tensor_tensor(T("sy0"), T("ry"), T("ZERO"), A.is_ge))
    V(lambda e: e.tensor_scalar(T("syp"), T("sy0"), 2.0, -1.0, A.mult, A.add))
    # projection center: r2 = rx^2+ry^2 (exact muls)
    V(lambda e: e.tensor_mul(T("sq1"), T("rx"), T("rx")))
    V(lambda e: e.tensor_mul(T("sq2"), T("ry"), T("ry")))
    V(lambda e: e.tensor_add(T("r2"), T("sq1"), T("sq2")))
    S(lambda e: e.activation(T("rr"), T("r2"), ACT.Sqrt))
    V(lambda e: e.tensor_tensor(T("rrc"), T("rr"), T("EPS"), A.max))
    V(lambda e: e.reciprocal(T("rinv"), T("rrc")))
    V(lambda e: e.tensor_scalar(T("cent"), T("ry"), 30.0, None, A.mult))
    V(lambda e: e.tensor_mul(T("cent2"), T("cent"), T("rinv")))
    V(lambda e: e.tensor_scalar(T("rc0"), T("cent2"), MAGIC, MAGIC, A.add, A.subtract))
    V(lambda e: e.tensor_tensor(T("rc1"), T("rc0"), T("CN28"), A.max))
    V(lambda e: e.tensor_tensor(T("rc"), T("rc1"), T("C28"), A.min))

    # ---------- sqrt arguments for the 6 xmax chains
    V(lambda e: e.tensor_mul(T("nx2"), T("rndx"), T("rndx")))
    V(lambda e: e.tensor_scalar(T("sA"), T("nx2"), -1.0, 900.0, A.mult, A.add))
    for dlt in (2, 1, 0, -1, -2):
        tg = f"w{dlt}"

        def mk(tg=tg, dlt=dlt):
            V(lambda e: e.tensor_scalar(T(tg + "_r"), T("rc"), float(dlt), None, A.add))
            V(lambda e: e.tensor_mul(T(tg + "_q"), T(tg + "_r"), T(tg + "_r")))
            V(lambda e: e.tensor_scalar(T(tg + "_s"), T(tg + "_q"), -1.0, 900.0, A.mult, A.add))
        mk()
    # 6 ACT sqrts
    S(lambda e: e.activation(T("A_y0"), T("sA"), ACT.Sqrt))
    for dlt in (2, 1, 0, -1, -2):
        tg = f"w{dlt}"
        S(lambda e, tg=tg: e.activation(T(tg + "_y0"), T(tg + "_s"), ACT.Sqrt))

    # exact xm = t - (t*t > s)
    def xm_fix(tg, s_name):
        V(lambda e: e.tensor_scalar(T(tg + "_t"), T(tg + "_y0"), MAGIC, MAGIC, A.add, A.subtract))
        V(lambda e: e.tensor_mul(T(tg + "_t2"), T(tg + "_t"), T(tg + "_t")))
        V(lambda e: e.tensor_tensor(T(tg + "_gt"), T(tg + "_t2"), T(s_name), A.is_gt))
        V(lambda e: e.tensor_sub(T(tg + "_xm"), T(tg + "_t"), T(tg + "_gt")))

    xm_fix("A", "sA")
    for dlt in (2, 1, 0, -1, -2):
        xm_fix(f"w{dlt}", f"w{dlt}_s")

    # ---------- candidates (gx_name, gy_name) in sim-validated order
    cands = []
    # A
    V(lambda e: e.tensor_scalar(T("nb"), T("A_xm"), -1.0, None, A.mult))
    V(lambda e: e.tensor_tensor(T("rA0"), T("rndy"), T("nb"), A.max))
    V(lambda e: e.tensor_tensor(T("rA"), T("rA0"), T("A_xm"), A.min))
    cands.append(("rndx", "rA"))
    # window rows
    for dlt in (2, 1, 0, -1, -2):
        tg = f"w{dlt}"

        def mk2(tg=tg):
            V(lambda e: e.tensor_tensor(T(tg + "_mn"), T("arnd"), T(tg + "_xm"), A.min))
            V(lambda e: e.tensor_mul(T(tg + "_gx"), T(tg + "_mn"), T("sxp")))
        mk2()
        cands.append((tg + "_gx", tg + "_r"))
    # hull vertices
    for hx, hy in HULL:
        tg = f"h{int(hx)}_{int(hy)}"

        def mk3(tg=tg, hx=hx, hy=hy):
            V(lambda e: e.tensor_scalar(T(tg + "_gx"), T("sxp"), hx, None, A.mult))
            V(lambda e: e.tensor_scalar(T(tg + "_gy"), T("syp"), hy, None, A.mult))
        mk3()
        cands.append((tg + "_gx", tg + "_gy"))

    # ---------- evaluate + running argmin (strict <, first wins ties)
    for k, (gxn, gyn) in enumerate(cands):
        tg = f"c{k}"

        def ev(tg=tg, gxn=gxn, gyn=gyn, k=k):
            V(lambda e: e.tensor_sub(T(tg + "_dx"), T("rx"), T(gxn)))
            V(lambda e: e.tensor_sub(T(tg + "_dy"), T("ry"), T(gyn)))
            V(lambda e: e.tensor_mul(T(tg + "_dx2"), T(tg + "_dx"), T(tg + "_dx")))
            V(lambda e: e.tensor_mul(T(tg + "_dy2"), T(tg + "_dy"), T(tg + "_dy")))
            V(lambda e: e.tensor_add(T(tg + "_d2"), T(tg + "_dx2"), T(tg + "_dy2")))
            if k == 0:
                V(lambda e: e.tensor_copy(T("best0"), T(tg + "_d2")))
                V(lambda e: e.tensor_copy(T("wgx"), T(gxn)))
                V(lambda e: e.tensor_copy(T("wgy"), T(gyn)))
            else:
                bprev = f"best{k-1}"; bnew = f"best{k}"
                V(lambda e, bprev=bprev: e.tensor_tensor(T(tg + "_m", dt=u8), T(tg + "_d2"), T(bprev), A.is_lt))
                V(lambda e, bprev=bprev, bnew=bnew: e.tensor_tensor(T(bnew), T(bprev), T(tg + "_d2"), A.min))
                V(lambda e: e.copy_predicated(T("wgx"), T(tg + "_m", dt=u8), T(gxn)))
                V(lambda e: e.copy_predicated(T("wgy"), T(tg + "_m", dt=u8), T(gyn)))
        ev()

    # ---------- offsets into packed output
    V(lambda e: e.tensor_sub(tiles["outt"][:, 0:F], T("rx"), T("wgx")))
    V(lambda e: e.tensor_sub(tiles["outt"][:, F:2 * F], T("ry"), T("wgy")))

    # ---------- index
    V(lambda e: e.tensor_scalar(T("irow"), T("wgy"), -1.0, 30.0, A.mult, A.add))
    V(lambda e: e.tensor_scalar(T("i2"), T("irow"), -1.0, 60.0, A.mult, A.add))
    V(lambda e: e.tensor_tensor(T("u"), T("irow"), T("i2"), A.min))
    V(lambda e: e.tensor_scalar(T("v"), T("u"), 1.0 / 15.0, -1.0, A.mult, A.add))
    V(lambda e: e.tensor_scalar(T("acc14"), T("v"), float(POLY[14]), None, A.mult))
    for kk in range(13, 0, -1):
        V(lambda e, kk=kk: e.scalar_tensor_tensor(T(f"acc{kk}"), T(f"acc{kk+1}"), float(POLY[kk]), T("v"), A.add, A.mult))
    V(lambda e: e.tensor_scalar(T("acc0"), T("acc1"), float(POLY[0]), None, A.add))
    V(lambda e: e.tensor_scalar(T("pb"), T("acc0"), MAGIC, MAGIC, A.add, A.subtract))
    # xmax of winner row
    V(lambda e: e.tensor_mul(T("wy2"), T("wgy"), T("wgy")))
    V(lambda e: e.tensor_scalar(T("sW"), T("wy2"), -1.0, 900.0, A.mult, A.add))
    S(lambda e: e.activation(T("W_y0"), T("sW"), ACT.Sqrt))
    xm_fix("W", "sW")
    # reflected base where irow > 30: 2820 - pb - 2*xm
    V(lambda e: e.scalar_tensor_tensor(T("t1a"), T("W_xm"), 2.0, T("pb"), A.mult, A.add))
    V(lambda e: e.tensor_scalar(T("t1"), T("t1a"), -1.0, 2820.0, A.mult, A.add))
    V(lambda e: e.tensor_tensor(T("mh", dt=u8), T("irow"), T("C30"), A.is_gt))
    V(lambda e: e.tensor_copy(T("base"), T("pb")))
    V(lambda e: e.copy_predicated(T("base"), T("mh", dt=u8), T("t1")))
    V(lambda e: e.tensor_add(T("idxf"), T("base"), T("W_xm")))
    V(lambda e: e.tensor_add(tiles["outt"][:, 2 * F:3 * F], T("idxf"), T("wgx")))

    # ---------- emit: serial ping-pong across V/S with semaphores
    segs = []          # (engine, [fns])
    for eng, fn in prog:
        if not segs or segs[-1][0] != eng:
            segs.append((eng, []))
        segs[-1][1].append(fn)
    nV = sum(1 for e, _ in segs if e == "V")

    with (
        nc.Block() as block,
        nc.semaphore("dma_sem") as dma_sem,
        nc.semaphore("dve_sem") as dve_sem,
        nc.semaphore("act_sem") as act_sem,
    ):
        @block.gpsimd
        def _(g):
            g.dma_start(out=inp, in_=inpd[:]).then_inc(dma_sem, 16)
            g.wait_ge(dve_sem, nV)
            g.dma_start(out=outd[:], in_=tiles["outt"][:]).then_inc(dma_sem, 16)
            g.wait_ge(dma_sem, 32)

        @block.vector
        def _(v):
            v.wait_ge(dma_sem, 16)
            lvl = 0
            for eng, fns in segs:
                if eng == "V":
                    for fn in fns:
                        ins = fn(v)
                        v.drain()
                    ins.then_inc(dve_sem, 1)
                else:
                    lvl += 1
                    v.wait_ge(act_sem, lvl)

        @block.scalar
        def _(s):
            lvl = 0
            for eng, fns in segs:
                if eng == "V":
                    lvl += 1
                    s.wait_ge(dve_sem, lvl)
                else:
                    for fn in fns:
                        ins = fn(s)
                        s.drain()
                    ins.then_inc(act_sem, 1)

    return nc


def kernel(x, y, theta_y, grid=None):
    """Full inputs -> full outputs (index int32 [N], offset [N,2])."""
    from concourse.bass_utils import run_bass_kernel_spmd

    key = "nc"
    if key not in _CACHE:
        _CACHE[key] = _build()
    nc = _CACHE[key]

    x = np.asarray(x, dtype=np.float32)
    y = np.asarray(y, dtype=np.float32)
    th = np.asarray(theta_y, dtype=np.float32)

    in_maps = []
    for cid in range(NCORES):
        sl = slice(cid * NS, (cid + 1) * NS)
        xs = x[sl].reshape(P, F, 2)
        ys = y[sl].reshape(P, F, 2)
        packed = np.concatenate(
            [xs[:, :, 0], xs[:, :, 1], ys[:, :, 0], ys[:, :, 1],
             th[sl].reshape(P, F)], axis=1)
        in_maps.append({"inp": np.ascontiguousarray(packed)})

    res = run_bass_kernel_spmd(nc, in_maps, list(range(NCORES)))
    outs = res.results

    idx = np.empty(N, dtype=np.int32)
    off = np.empty((N, 2), dtype=np.float32)
    for cid in range(NCORES):
        sl = slice(cid * NS, (cid + 1) * NS)
        o = np.asarray(outs[cid]["outp"]).reshape(P, 3 * F)
        off[sl, 0] = o[:, 0:F].reshape(NS)
        off[sl, 1] = o[:, F:2 * F].reshape(NS)
        idx[sl] = np.rint(o[:, 2 * F:3 * F].reshape(NS)).astype(np.int32)
    return idx, off


# revision 24
# speedup vs baseline: 1.0056x; 1.0056x over previous
"""Trainium2 Bass kernel for nn_Attr_Tokenizer (retrieval_knn).

For N=131072 samples: center (x-y), rotate into the anchor frame, find the
nearest of the 2821 fixed disk-grid points, return (index int32 [N],
offset [N,2]).

Exact closed form (no 2821-wide scan). Per sample, 12 candidates provably
cover the argmin (validated on 10M-sample sweeps + the real data):
  A     = (round(rx), clamp(round(ry), +-xmax(round(rx))))   [interior opt]
  rows  = 5 rows around the disk-projection row; per-row optimum is
          (sign(rx)*min(|round(rx)|, xmax(r)), r)
  hull  = 6 convex-hull vertices of the lattice-disk boundary (mirrored)
xmax(r) = floor(sqrt(900-r^2)) computed exactly: t = round(ACT_sqrt(s));
xm = t - (t*t > s)  -- tolerant of the loose ACT sqrt LUT.
index = base[i] + xmax + gx with i = 30-gy; base[] evaluated by an
exact-rounding degree-14 polynomial on the half domain + reflection.

fp32 simulation vs the jax reference: 1/131072 index mismatch (an fp32
sqrt-collapse tie), offset rel err ~1e-3.

Implementation: raw Bass (no Tile) - serial engine ping-pong with
single-semaphore waits. Data-parallel over 8 NeuronCores (shard N,
no cross-core communication).
"""
import numpy as np

N = 131072
NCORES = 8
NS = N // NCORES       # 16384 per core
P = 128
F = NS // P            # 128

MAGIC = 12582912.0     # 1.5*2^23: (x+M)-M == round-to-nearest for |x| < 2^22

# deg-14 power-basis coefficients in v=(u-15)/15 for base[u], u=0..30
# (discrete minimax fit, max residual 0.441; fp32 Horner + round is exact)
POLY = [526.652099609375, 767.9715576171875, 131.24649047851562,
        27.565488815307617, 32.190269470214844, -676.077880859375,
        -417.0867004394531, 2375.08642578125, 2140.177734375,
        -4234.26904296875, -4233.40185546875, 3640.740234375,
        3674.610107421875, -1211.4583740234375, -1164.38818359375]

HULL = [(29.0, 7.0), (27.0, 13.0), (24.0, 18.0),
        (18.0, 24.0), (13.0, 27.0), (7.0, 29.0)]

_CACHE = {}


def _build():
    import concourse.bass as bass
    from concourse import mybir

    f32 = mybir.dt.float32
    u8 = mybir.dt.uint8
    A = mybir.AluOpType
    ACT = mybir.ActivationFunctionType
    HALFPI = float(np.pi / 2)

    nc = bass.Bass()
    inpd = nc.declare_dram_parameter("inp", [P, 5 * F], f32, isOutput=False)
    outd = nc.declare_dram_parameter("outp", [P, 3 * F], f32, isOutput=True)

    tiles = {}

    def T(name, w=F, dt=None):
        if name not in tiles:
            tiles[name] = nc.alloc_sbuf_tensor(name, [P, w], dt or f32)
        return tiles[name][:]

    inp = T("inp_t", 5 * F)
    outt = T("outt", 3 * F)
    x0 = tiles["inp_t"][:, 0:F]
    x1 = tiles["inp_t"][:, F:2 * F]
    y0 = tiles["inp_t"][:, 2 * F:3 * F]
    y1 = tiles["inp_t"][:, 3 * F:4 * F]
    th = tiles["inp_t"][:, 4 * F:5 * F]

    # linear program: (engine, fn, drain) steps; engines alternate with a
    # semaphore ping-pong so every instruction carries <= 1 wait. Drains are
    # emitted only where an op reads the IMMEDIATELY preceding DVE op's
    # output (the raw-bass RAW hazard window is ~1 instruction deep; ops
    # >=2 apart are separated by a full 128-element stream).
    prog = []  # ("V"|"S", callable(engine), drain_after: bool)

    def V(fn):
        prog.append(("V", fn, False))

    def Vd(fn):
        prog.append(("V", fn, True))

    def S(fn):
        prog.append(("S", fn, False))

    # ---------- constant tiles (no consumers nearby)
    V(lambda e: e.memset(T("ZERO"), 0.0))
    V(lambda e: e.memset(T("C30"), 30.0))
    V(lambda e: e.memset(T("CN30"), -30.0))
    V(lambda e: e.memset(T("C28"), 28.0))
    V(lambda e: e.memset(T("CN28"), -28.0))
    V(lambda e: e.memset(T("EPS"), 1e-6))
    V(lambda e: e.memset(T("CPI"), float(np.pi)))

    # ---------- center + wrapped theta (interleaved to avoid drains)
    V(lambda e: e.tensor_scalar(T("th2"), th, HALFPI, None, A.add))
    V(lambda e: e.tensor_sub(T("cx0"), x0, y0))
    V(lambda e: e.tensor_tensor(T("thm", dt=u8), T("th2"), T("CPI"), A.is_gt))
    V(lambda e: e.tensor_sub(T("cy0"), x1, y1))
    V(lambda e: e.tensor_copy(T("thmf"), T("thm", dt=u8)))
    Vd(lambda e: e.scalar_tensor_tensor(T("th2w"), T("thmf"), float(-2.0 * np.pi), T("th2"), A.mult, A.add))
    # ---------- sin/cos (ACT)
    S(lambda e: e.activation(T("c"), th, ACT.Sin))
    S(lambda e: e.activation(T("s"), T("th2w"), ACT.Sin))
    # ---------- rotate (same fp32 op order as the reference)
    V(lambda e: e.tensor_mul(T("m1"), T("cx0"), T("c")))
    V(lambda e: e.tensor_mul(T("m2"), T("cy0"), T("s")))
    V(lambda e: e.tensor_mul(T("m3"), T("cx0"), T("s")))
    V(lambda e: e.tensor_mul(T("m4"), T("cy0"), T("c")))
    V(lambda e: e.tensor_sub(T("rx"), T("m1"), T("m2")))
    V(lambda e: e.tensor_add(T("ry"), T("m3"), T("m4")))
    # rounds / abs / signs / squares, interleaved for >=2-op reuse distance
    V(lambda e: e.tensor_scalar(T("rnd0"), T("rx"), MAGIC, MAGIC, A.add, A.subtract))
    V(lambda e: e.tensor_scalar(T("rndy"), T("ry"), MAGIC, MAGIC, A.add, A.subtract))
    V(lambda e: e.tensor_tensor(T("rnd1"), T("rnd0"), T("CN30"), A.max))
    V(lambda e: e.tensor_tensor(T("sx0"), T("rx"), T("ZERO"), A.is_ge))
    V(lambda e: e.tensor_tensor(T("rndx"), T("rnd1"), T("C30"), A.min))
    V(lambda e: e.tensor_tensor(T("sy0"), T("ry"), T("ZERO"), A.is_ge))
    V(lambda e: e.tensor_scalar(T("nrndx"), T("rndx"), -1.0, None, A.mult))
    V(lambda e: e.tensor_scalar(T("sxp"), T("sx0"), 2.0, -1.0, A.mult, A.add))
    V(lambda e: e.tensor_tensor(T("arnd"), T("rndx"), T("nrndx"), A.max))
    V(lambda e: e.tensor_scalar(T("syp"), T("sy0"), 2.0, -1.0, A.mult, A.add))
    V(lambda e: e.tensor_mul(T("sq1"), T("rx"), T("rx")))
    V(lambda e: e.tensor_mul(T("sq2"), T("ry"), T("ry")))
    V(lambda e: e.tensor_mul(T("nx2"), T("rndx"), T("rndx")))
    V(lambda e: e.tensor_add(T("r2"), T("sq1"), T("sq2")))
    Vd(lambda e: e.tensor_scalar(T("sA"), T("nx2"), -1.0, 900.0, A.mult, A.add))
    # center sqrt
    S(lambda e: e.activation(T("rr"), T("r2"), ACT.Sqrt))
    # while ACT runs nothing else is ready; continue after wait
    V(lambda e: e.tensor_tensor(T("rrc"), T("rr"), T("EPS"), A.max))
    Vd(lambda e: e.reciprocal(T("rinv"), T("rrc")))
    V(lambda e: e.tensor_scalar(T("cent"), T("ry"), 30.0, None, A.mult))
    Vd(lambda e: e.tensor_mul(T("cent2"), T("cent"), T("rinv")))
    Vd(lambda e: e.tensor_scalar(T("rc0"), T("cent2"), MAGIC, MAGIC, A.add, A.subtract))
    Vd(lambda e: e.tensor_tensor(T("rc1"), T("rc0"), T("CN28"), A.max))
    Vd(lambda e: e.tensor_tensor(T("rc"), T("rc1"), T("C28"), A.min))
    # ---------- window-row sqrt args (batched: no adjacent reuse)
    for dlt in (2, 1, 0, -1, -2):
        V(lambda e, tg=f"w{dlt}", dlt=dlt: e.tensor_scalar(T(tg + "_r"), T("rc"), float(dlt), None, A.add))
    for dlt in (2, 1, 0, -1, -2):
        V(lambda e, tg=f"w{dlt}": e.tensor_mul(T(tg + "_q"), T(tg + "_r"), T(tg + "_r")))
    for dlt in (2, 1, 0, -1, -2):
        V(lambda e, tg=f"w{dlt}": e.tensor_scalar(T(tg + "_s"), T(tg + "_q"), -1.0, 900.0, A.mult, A.add))
    # ---------- 6 ACT sqrts (A-band + 5 rows)
    S(lambda e: e.activation(T("A_y0"), T("sA"), ACT.Sqrt))
    for dlt in (2, 1, 0, -1, -2):
        S(lambda e, tg=f"w{dlt}": e.activation(T(tg + "_y0"), T(tg + "_s"), ACT.Sqrt))

    # ---------- hull candidates evaluated while ACT sqrts run
    HT = [f"h{int(hx)}_{int(hy)}" for hx, hy in HULL]
    for (hx, hy), tg in zip(HULL, HT):
        V(lambda e, tg=tg, hx=hx: e.tensor_scalar(T(tg + "_gx"), T("sxp"), hx, None, A.mult))
        V(lambda e, tg=tg, hy=hy: e.tensor_scalar(T(tg + "_gy"), T("syp"), hy, None, A.mult))
    for tg in HT:
        V(lambda e, tg=tg: e.tensor_sub(T(tg + "_dx"), T("rx"), T(tg + "_gx")))
        V(lambda e, tg=tg: e.tensor_sub(T(tg + "_dy"), T("ry"), T(tg + "_gy")))
    for tg in HT:
        V(lambda e, tg=tg: e.tensor_mul(T(tg + "_dx2"), T(tg + "_dx"), T(tg + "_dx")))
        V(lambda e, tg=tg: e.tensor_mul(T(tg + "_dy2"), T(tg + "_dy"), T(tg + "_dy")))
    for tg in HT:
        V(lambda e, tg=tg: e.tensor_add(T(tg + "_d2"), T(tg + "_dx2"), T(tg + "_dy2")))

    # ---------- xm fix chains, interleaved across the 6 chains (no drains)
    XT = ["A"] + [f"w{dlt}" for dlt in (2, 1, 0, -1, -2)]
    for tg in XT:
        V(lambda e, tg=tg: e.tensor_scalar(T(tg + "_t"), T(tg + "_y0"), MAGIC, MAGIC, A.add, A.subtract))
    for tg in XT:
        V(lambda e, tg=tg: e.tensor_mul(T(tg + "_t2"), T(tg + "_t"), T(tg + "_t")))
    for tg in XT:
        sn = "sA" if tg == "A" else tg + "_s"
        V(lambda e, tg=tg, sn=sn: e.tensor_tensor(T(tg + "_gt"), T(tg + "_t2"), T(sn), A.is_gt))
    for tg in XT:
        V(lambda e, tg=tg: e.tensor_sub(T(tg + "_xm"), T(tg + "_t"), T(tg + "_gt")))

    # ---------- A + row candidate coordinates (interleaved)
    V(lambda e: e.tensor_scalar(T("nb"), T("A_xm"), -1.0, None, A.mult))
    for dlt in (2, 1, 0, -1, -2):
        V(lambda e, tg=f"w{dlt}": e.tensor_tensor(T(tg + "_mn"), T("arnd"), T(tg + "_xm"), A.min))
    V(lambda e: e.tensor_tensor(T("rA0"), T("rndy"), T("nb"), A.max))
    for dlt in (2, 1, 0, -1, -2):
        V(lambda e, tg=f"w{dlt}": e.tensor_mul(T(tg + "_gx"), T(tg + "_mn"), T("sxp")))
    Vd(lambda e: e.tensor_tensor(T("rA"), T("rA0"), T("A_xm"), A.min))

    # ---------- A + row evals (batched)
    cands = [("rndx", "rA")] + [(f"w{dlt}_gx", f"w{dlt}_r") for dlt in (2, 1, 0, -1, -2)]
    AT = [f"c{k}" for k in range(6)]
    for (gxn, gyn), tg in zip(cands, AT):
        V(lambda e, tg=tg, gxn=gxn: e.tensor_sub(T(tg + "_dx"), T("rx"), T(gxn)))
        V(lambda e, tg=tg, gyn=gyn: e.tensor_sub(T(tg + "_dy"), T("ry"), T(gyn)))
    for tg in AT:
        V(lambda e, tg=tg: e.tensor_mul(T(tg + "_dx2"), T(tg + "_dx"), T(tg + "_dx")))
        V(lambda e, tg=tg: e.tensor_mul(T(tg + "_dy2"), T(tg + "_dy"), T(tg + "_dy")))
    for tg in AT:
        V(lambda e, tg=tg: e.tensor_add(T(tg + "_d2"), T(tg + "_dx2"), T(tg + "_dy2")))

    # ---------- running argmin in semantic order (A, rows +2..-2, hull)
    order = AT + HT
    gxy = cands + [(tg + "_gx", tg + "_gy") for tg in HT]
    for k, (tg, (gxn, gyn)) in enumerate(zip(order, gxy)):
        if k == 0:
            V(lambda e, tg=tg: e.tensor_copy(T("best0"), T(tg + "_d2")))
            V(lambda e, gxn=gxn: e.tensor_copy(T("wgx"), T(gxn)))
            V(lambda e, gyn=gyn: e.tensor_copy(T("wgy"), T(gyn)))
        else:
            bprev, bnew = f"best{k-1}", f"best{k}"
            V(lambda e, tg=tg, bprev=bprev: e.tensor_tensor(T(tg + "_m", dt=u8), T(tg + "_d2"), T(bprev), A.is_lt))
            V(lambda e, tg=tg, bprev=bprev, bnew=bnew: e.tensor_tensor(T(bnew), T(bprev), T(tg + "_d2"), A.min))
            V(lambda e, tg=tg, gxn=gxn: e.copy_predicated(T("wgx"), T(tg + "_m", dt=u8), T(gxn)))
            V(lambda e, tg=tg, gyn=gyn: e.copy_predicated(T("wgy"), T(tg + "_m", dt=u8), T(gyn)))

    # ---------- offsets + index front (interleaved)
    V(lambda e: e.tensor_sub(tiles["outt"][:, 0:F], T("rx"), T("wgx")))
    V(lambda e: e.tensor_sub(tiles["outt"][:, F:2 * F], T("ry"), T("wgy")))
    V(lambda e: e.tensor_scalar(T("irow"), T("wgy"), -1.0, 30.0, A.mult, A.add))
    V(lambda e: e.tensor_mul(T("wy2"), T("wgy"), T("wgy")))
    V(lambda e: e.tensor_scalar(T("i2"), T("irow"), -1.0, 60.0, A.mult, A.add))
    V(lambda e: e.tensor_scalar(T("sW"), T("wy2"), -1.0, 900.0, A.mult, A.add))
    V(lambda e: e.tensor_tensor(T("u"), T("irow"), T("i2"), A.min))
    Vd(lambda e: e.tensor_scalar(T("v"), T("u"), 1.0 / 15.0, -1.0, A.mult, A.add))
    S(lambda e: e.activation(T("W_y0"), T("sW"), ACT.Sqrt))
    # poly chain interleaved with the W xm-fix chain
    V(lambda e: e.tensor_scalar(T("acc14"), T("v"), float(POLY[14]), None, A.mult))
    V(lambda e: e.tensor_scalar(T("W_t"), T("W_y0"), MAGIC, MAGIC, A.add, A.subtract))
    V(lambda e: e.scalar_tensor_tensor(T("acc13"), T("acc14"), float(POLY[13]), T("v"), A.add, A.mult))
    V(lambda e: e.tensor_mul(T("W_t2"), T("W_t"), T("W_t")))
    V(lambda e: e.scalar_tensor_tensor(T("acc12"), T("acc13"), float(POLY[12]), T("v"), A.add, A.mult))
    V(lambda e: e.tensor_tensor(T("W_gt"), T("W_t2"), T("sW"), A.is_gt))
    V(lambda e: e.scalar_tensor_tensor(T("acc11"), T("acc12"), float(POLY[11]), T("v"), A.add, A.mult))
    V(lambda e: e.tensor_sub(T("W_xm"), T("W_t"), T("W_gt")))
    for kk in range(10, 0, -1):
        Vd(lambda e, kk=kk: e.scalar_tensor_tensor(T(f"acc{kk}"), T(f"acc{kk+1}"), float(POLY[kk]), T("v"), A.add, A.mult))
    Vd(lambda e: e.tensor_scalar(T("acc0"), T("acc1"), float(POLY[0]), None, A.add))
    Vd(lambda e: e.tensor_scalar(T("pb"), T("acc0"), MAGIC, MAGIC, A.add, A.subtract))
    # reflected base where irow > 30: 2820 - pb - 2*xm
    V(lambda e: e.scalar_tensor_tensor(T("t1a"), T("W_xm"), 2.0, T("pb"), A.mult, A.add))
    V(lambda e: e.tensor_tensor(T("mh", dt=u8), T("irow"), T("C30"), A.is_gt))
    V(lambda e: e.tensor_scalar(T("t1"), T("t1a"), -1.0, 2820.0, A.mult, A.add))
    V(lambda e: e.tensor_copy(T("base"), T("pb")))
    Vd(lambda e: e.copy_predicated(T("base"), T("mh", dt=u8), T("t1")))
    Vd(lambda e: e.tensor_add(T("idxf"), T("base"), T("W_xm")))
    Vd(lambda e: e.tensor_add(tiles["outt"][:, 2 * F:3 * F], T("idxf"), T("wgx")))

    # ---------- emit: serial ping-pong across V/S with semaphores
    segs = []          # (engine, [(fn, drain)])
    for eng, fn, dr in prog:
        if not segs or segs[-1][0] != eng:
            segs.append((eng, []))
        segs[-1][1].append((fn, dr))
    nV = sum(1 for e, _ in segs if e == "V")

    with (
        nc.Block() as block,
        nc.semaphore("dma_sem") as dma_sem,
        nc.semaphore("dve_sem") as dve_sem,
        nc.semaphore("act_sem") as act_sem,
    ):
        @block.gpsimd
        def _(g):
            g.dma_start(out=inp, in_=inpd[:]).then_inc(dma_sem, 16)
            g.wait_ge(dve_sem, nV)
            g.dma_start(out=outd[:], in_=tiles["outt"][:]).then_inc(dma_sem, 16)
            g.wait_ge(dma_sem, 32)

        @block.vector
        def _(v):
            v.wait_ge(dma_sem, 16)
            lvl = 0
            for eng, fns in segs:
                if eng == "V":
                    for fn, dr in fns:
                        fn(v)
                        if dr:
                            v.drain()
                    v.drain().then_inc(dve_sem, 1)
                else:
                    lvl += 1
                    v.wait_ge(act_sem, lvl)

        @block.scalar
        def _(s):
            lvl = 0
            for eng, fns in segs:
                if eng == "V":
                    lvl += 1
                    s.wait_ge(dve_sem, lvl)
                else:
                    for fn, dr in fns:
                        fn(s)
                    s.drain().then_inc(act_sem, 1)

    return nc


def _get_runner():
    """Build the sharded PJRT executable ONCE (bass2jax re-traces per call)."""
    if "runner" in _CACHE:
        return _CACHE["runner"]
    import jax
    from concourse import bass2jax, mybir

    if "nc" not in _CACHE:
        _CACHE["nc"] = _build()
    nc = _CACHE["nc"]
    bass2jax.install_neuronx_cc_hook()

    # derive I/O metadata exactly as bass2jax.run_bass_via_pjrt does
    in_names = []
    out_names = []
    out_avals = []
    for alloc in nc.m.functions[0].allocations:
        if not isinstance(alloc, mybir.MemoryLocationSet):
            continue
        if not alloc.memorylocations:
            continue
        name = alloc.memorylocations[0].name
        if alloc.kind == "ExternalInput":
            in_names.append(name)
        elif alloc.kind == "ExternalOutput":
            out_names.append(name)
            out_avals.append(jax.core.ShapedArray(
                tuple(alloc.tensor_shape), mybir.dt.np(alloc.dtype)))
    n_params = len(in_names)
    in_names = in_names + out_names

    def _body(*args):
        outs = bass2jax._bass_exec_p.bind(
            *args,
            out_avals=tuple(out_avals),
            in_names=tuple(in_names),
            out_names=tuple(out_names),
            lowering_input_output_aliases=(),
            sim_require_finite=True,
            sim_require_nnan=True,
            nc=nc,
        )
        return tuple(outs)

    devices = jax.devices()[:NCORES]
    mesh = bass2jax.Mesh(np.asarray(devices), ("core",))
    nin = n_params + len(out_names)
    sharded = jax.jit(
        bass2jax.shard_map(
            _body, mesh=mesh,
            in_specs=(bass2jax.PartitionSpec("core"),) * nin,
            out_specs=(bass2jax.PartitionSpec("core"),) * len(out_names),
            check_rep=False,
        ),
        donate_argnums=tuple(range(n_params, nin)),
        keep_unused=True,
    )
    _CACHE["runner"] = sharded
    return sharded


def kernel(x, y, theta_y, grid=None):
    """Full inputs -> full outputs (index int32 [N], offset [N,2])."""
    x = np.asarray(x, dtype=np.float32)
    y = np.asarray(y, dtype=np.float32)
    th = np.asarray(theta_y, dtype=np.float32)

    # pack [x0|x1|y0|y1|th] per core, concat on axis 0 for shard_map
    xs = x.reshape(NCORES, P, F, 2)
    ys = y.reshape(NCORES, P, F, 2)
    ths = th.reshape(NCORES, P, F)
    packed = np.ascontiguousarray(np.concatenate(
        [xs[..., 0], xs[..., 1], ys[..., 0], ys[..., 1], ths], axis=2
    ).reshape(NCORES * P, 5 * F))

    o = None
    try:
        sharded = _get_runner()
        zeros = np.zeros((NCORES * P, 3 * F), np.float32)
        (out_arr,) = sharded(packed, zeros)
        o = np.asarray(out_arr).reshape(NCORES, P, 3 * F)
    except Exception:
        o = None
    if o is None:
        # fallback: library dispatch path (re-traces per call, slower)
        from concourse.bass_utils import run_bass_kernel_spmd
        if "nc" not in _CACHE:
            _CACHE["nc"] = _build()
        in_maps = [{"inp": np.ascontiguousarray(packed[c * P:(c + 1) * P])}
                   for c in range(NCORES)]
        res = run_bass_kernel_spmd(_CACHE["nc"], in_maps, list(range(NCORES)))
        o = np.stack([np.asarray(res.results[c]["outp"]).reshape(P, 3 * F)
                      for c in range(NCORES)])

    off = np.empty((N, 2), dtype=np.float32)
    off[:, 0] = o[:, :, 0:F].reshape(N)
    off[:, 1] = o[:, :, F:2 * F].reshape(N)
    idx = np.rint(o[:, :, 2 * F:3 * F].reshape(N)).astype(np.int32)
    return idx, off
